# revision 12
# baseline (speedup 1.0000x reference)
"""Trainium2 Bass kernel for causal self-attention with doubled rotary.

Full-input contract: kernel(**inputs) takes the complete tensors
(x [4,2048,2048], wq/wk/wv/wo [2048,2048]) and returns [4,2048,2048] fp32.

Sharding: 8 cores = 4 batch elements x 2 head-halves (8 heads each).
Each core computes a partial output projection (its heads' columns of wo);
the host sums the two partials per batch element.

Math notes:
  - The reference applies rotary twice; R(theta)^2 == R(2*theta), so we
    apply a single rotation with doubled-angle tables.
  - Attention is computed transposed: ST[s,t] = sum_d kT[d,s] qT[d,t].
    exp(ST) feeds the PV matmul directly (lhsT = v in natural [s,d] layout),
    so no on-chip transposes are needed. Softmax denominators are per-t
    column sums of exp(ST): accumulated per s-chunk on DVE, partition-reduced
    with a ones-vector matmul, reciprocal on DVE, broadcast on GpSimd, and
    applied to the PV output during the PSUM->SBUF copy.
  - Matmuls run in float32r mode (fp32 bits, fast PE path).
"""

import os
import sys

for _p in ("/opt/trn_rl_repo", "/root/.axon_site/_ro/trn_rl_repo"):
    if os.path.isdir(_p) and _p not in sys.path:
        sys.path.insert(0, _p)

import numpy as np

import concourse.bass as bass
import concourse.mybir as mybir
from concourse import bacc
from concourse.bass import ds
from concourse.tile import TileContext
from concourse.bass_utils import run_bass_kernel_spmd

F32 = mybir.dt.float32
F32R = mybir.dt.float32r

P = 128          # partitions / head dim
T = 2048         # sequence length
E = 2048         # embedding dim
B = 4
N_HEAD = 16
HPC = 8          # heads per core
D = 128          # head dim
PAN = 512        # t-panel width (PSUM bank limit for fp32)
NPAN = T // PAN  # 4
EO = E // P      # 16 contraction chunks for projections
NGRP = 4         # head pairs per core
NCH_TILES = T // P  # 16 s-chunks (also v t-tiles)
SCALE = 1.0 / float(np.sqrt(D))
NEG = -1.0e9

def build_program():
    nc = bacc.Bacc()

    xT = nc.declare_dram_parameter("xT", [E, T], F32R, isOutput=False)           # x[b].T
    wqT = nc.declare_dram_parameter("wqT", [E, HPC * D], F32R, isOutput=False)   # wq[rows].T
    wkT = nc.declare_dram_parameter("wkT", [E, HPC * D], F32R, isOutput=False)
    wvT = nc.declare_dram_parameter("wvT", [E, HPC * D], F32R, isOutput=False)
    woT = nc.declare_dram_parameter("woT", [HPC * D, E], F32R, isOutput=False)   # wo[:,cols].T
    cos2 = nc.declare_dram_parameter("cos2", [P, T], F32, isOutput=False)
    sin2 = nc.declare_dram_parameter("sin2", [P, T], F32, isOutput=False)
    masks = nc.declare_dram_parameter("masks", [P, PAN + 384], F32, isOutput=False)
    out = nc.declare_dram_parameter("out", [E, T], F32, isOutput=True)
    ytd = nc.dram_tensor("yt_scratch", [HPC * D, T], F32R)       # yT spill

    with TileContext(nc) as tc:
        with tc.tile_pool(name="const", bufs=1) as cpool:
            c2 = cpool.tile([P, T], F32, tag="c2")
            nc.sync.dma_start(c2, cos2[:, :])
            s2 = cpool.tile([P, T], F32, tag="s2")
            nc.sync.dma_start(s2, sin2[:, :])
            mk = cpool.tile([P, PAN + 384], F32, tag="mk")
            nc.sync.dma_start(mk, masks[:, :])
            ones_f = cpool.tile([P, 1], F32, tag="ones_f")
            nc.vector.memset(ones_f, 1.0)
            ones = cpool.tile([P, 1], F32R, tag="ones")
            nc.scalar.copy(ones, ones_f)
            ones1_f = cpool.tile([1, P], F32, tag="ones1_f")
            nc.vector.memset(ones1_f, 1.0)
            ones1 = cpool.tile([1, P], F32R, tag="ones1")
            nc.scalar.copy(ones1, ones1_f)

            _emit_attention(nc, tc, xT, wqT, wkT, wvT, ytd, c2, s2, mk, ones, ones1)
            _emit_output_proj(nc, tc, woT, ytd, out)

    nc.finalize()
    return nc


def _emit_attention(nc, tc, xT, wqT, wkT, wvT, ytd, c2, s2, mk, ones, ones1):
    with (
        tc.tile_pool(name="xp", bufs=2) as xpool,
        tc.tile_pool(name="wp", bufs=1) as wpool,
        tc.tile_pool(name="qk", bufs=1) as qkpool,
        tc.tile_pool(name="vp", bufs=1) as vpool,
        tc.tile_pool(name="rot", bufs=2) as rotpool,
        tc.tile_pool(name="sw", bufs=2) as swpool,
        tc.tile_pool(name="ex", bufs=2) as expool,
        tc.tile_pool(name="dn", bufs=2) as dnpool,
        tc.tile_pool(name="dn1", bufs=1) as dn1pool,
        tc.tile_pool(name="yts", bufs=2) as ytpool,
        tc.tile_pool(name="psA", bufs=2, space="PSUM") as psA,
        tc.tile_pool(name="psS", bufs=2, space="PSUM") as psS,
        tc.tile_pool(name="psY", bufs=2, space="PSUM") as psY,
        tc.tile_pool(name="psD", bufs=1, space="PSUM") as psD,
    ):
        for g in range(NGRP):
            # ---- projections for heads (2g, 2g+1) ----
            wq_sb = wpool.tile([P, EO, 2 * D], F32R, tag="wq")
            nc.sync.dma_start(
                wq_sb, wqT.rearrange("(eo p) d -> p eo d", p=P)[:, :, ds(g * 2 * D, 2 * D)]
            )
            wk_sb = wpool.tile([P, EO, 2 * D], F32R, tag="wk")
            nc.sync.dma_start(
                wk_sb, wkT.rearrange("(eo p) d -> p eo d", p=P)[:, :, ds(g * 2 * D, 2 * D)]
            )
            wv_sb = wpool.tile([P, EO, 2 * D], F32R, tag="wv")
            nc.sync.dma_start(
                wv_sb, wvT.rearrange("(eo p) d -> p eo d", p=P)[:, :, ds(g * 2 * D, 2 * D)]
            )

            qT = qkpool.tile([P, 2, T], F32R, tag="qT")
            kT = qkpool.tile([P, 2, T], F32R, tag="kT")
            v_sb = vpool.tile([P, NCH_TILES, 2 * D], F32R, tag="v")

            for jp in range(NPAN):
                xp = xpool.tile([P, EO, PAN], F32R, tag="xp")
                nc.sync.dma_start(
                    xp, xT.rearrange("(eo p) t -> p eo t", p=P)[:, :, ds(jp * PAN, PAN)]
                )
                # qT / kT panels: out [d 128, t 512], contract over e
                for w_sb, dst in ((wq_sb, qT), (wk_sb, kT)):
                    for hl in range(2):
                        ps = psA.tile([P, PAN], F32, tag="psA")
                        for eo in range(EO):
                            nc.tensor.matmul(
                                ps,
                                lhsT=w_sb[:, eo, ds(hl * D, D)],
                                rhs=xp[:, eo, :],
                                start=(eo == 0),
                                stop=(eo == EO - 1),
                            )
                        nc.scalar.copy(dst[:, hl, ds(jp * PAN, PAN)], ps)
                # v panels: out [t 128, d 256], contract over e
                for tt in range(PAN // P):
                    ps = psA.tile([P, PAN], F32, tag="psA")
                    psv = ps[:, : 2 * D]
                    for eo in range(EO):
                        nc.tensor.matmul(
                            psv,
                            lhsT=xp[:, eo, ds(tt * P, P)],
                            rhs=wv_sb[:, eo, :],
                            start=(eo == 0),
                            stop=(eo == EO - 1),
                        )
                    nc.scalar.copy(v_sb[:, jp * (PAN // P) + tt, :], psv)

            # ---- rotary (doubled angle) on qT, kT, in place ----
            for src in (qT, kT):
                for hl in range(2):
                    for jp in range(NPAN):
                        sl = ds(jp * PAN, PAN)
                        qsw = swpool.tile([P, PAN], F32R, tag="qsw")
                        nc.sync.dma_start(qsw[0:64, :], src[64:128, hl, sl])
                        nc.sync.dma_start(qsw[64:128, :], src[0:64, hl, sl])
                        tmp = rotpool.tile([P, PAN], F32, tag="rtmp")
                        nc.vector.tensor_tensor(
                            tmp, qsw[:, :], s2[:, sl], op=mybir.AluOpType.mult
                        )
                        nc.vector.tensor_tensor(
                            src[:, hl, sl], src[:, hl, sl], c2[:, sl],
                            op=mybir.AluOpType.mult,
                        )
                        nc.vector.tensor_tensor(
                            src[:, hl, sl], src[:, hl, sl], tmp,
                            op=mybir.AluOpType.add,
                        )

            # ---- attention for the two heads ----
            for hl in range(2):
                hc = 2 * g + hl  # head index within this core
                for jp in range(NPAN):
                    nch = 4 * jp + 4  # causal: s-chunks 0 .. 4*jp+3
                    ytp = psY.tile([P, PAN], F32, tag="psY")
                    den = dnpool.tile([P, PAN], F32, tag="den")
                    for i in range(nch):
                        st = psS.tile([P, PAN], F32, tag="psS")
                        nc.tensor.matmul(
                            st,
                            lhsT=kT[:, hl, ds(i * P, P)],
                            rhs=qT[:, hl, ds(jp * PAN, PAN)],
                            start=True,
                            stop=True,
                        )
                        di = i - 4 * jp
                        if di >= 0:
                            nc.vector.tensor_tensor(
                                st, st, mk[:, ds(384 - 128 * di, PAN)],
                                op=mybir.AluOpType.add,
                            )
                        ex = expool.tile([P, PAN], F32R, tag="ex")
                        nc.scalar.activation(
                            ex, st, mybir.ActivationFunctionType.Exp, scale=SCALE
                        )
                        if i == 0:
                            nc.vector.tensor_copy(den, ex)
                        else:
                            nc.vector.tensor_tensor(
                                den, den, ex, op=mybir.AluOpType.add
                            )
                        nc.tensor.matmul(
                            ytp,
                            lhsT=v_sb[:, i, ds(hl * D, D)],
                            rhs=ex,
                            start=(i == 0),
                            stop=(i == nch - 1),
                        )
                    # denominator: partition-reduce, reciprocal, broadcast
                    denr = dn1pool.tile([P, PAN], F32R, tag="denr")
                    nc.scalar.copy(denr, den)
                    dps = psD.tile([1, PAN], F32, tag="psD")
                    nc.tensor.matmul(
                        dps, lhsT=ones, rhs=denr, start=True, stop=True
                    )
                    rden = dn1pool.tile([1, PAN], F32R, tag="rden")
                    with nc.allow_low_precision(reason="f32r rounding of 1/den"):
                        nc.vector.reciprocal(rden, dps)
                    rdbp = psD.tile([P, PAN], F32, tag="rdbp")
                    nc.tensor.matmul(
                        rdbp, lhsT=ones1, rhs=rden, start=True, stop=True
                    )
                    rdb = dn1pool.tile([P, PAN], F32, tag="rdb")
                    nc.scalar.copy(rdb, rdbp)
                    yts = ytpool.tile([P, PAN], F32R, tag="yts")
                    nc.vector.tensor_tensor(
                        yts, ytp, rdb, op=mybir.AluOpType.mult
                    )
                    nc.sync.dma_start(
                        ytd[ds(hc * D, D), ds(jp * PAN, PAN)], yts
                    )


def _emit_output_proj(nc, tc, woT, ytd, out):
    with (
        tc.tile_pool(name="wo", bufs=1) as wopool,
        tc.tile_pool(name="yl", bufs=2) as ylpool,
        tc.tile_pool(name="ob", bufs=3) as opool,
        tc.tile_pool(name="psO", bufs=2, space="PSUM") as psO,
    ):
        wo_sb = wopool.tile([P, HPC, E], F32R, tag="wo")
        nc.sync.dma_start(wo_sb, woT.rearrange("(c p) e -> p c e", p=P))
        for jp in range(NPAN):
            yl = ylpool.tile([P, HPC, PAN], F32R, tag="yl")
            nc.sync.dma_start(
                yl, ytd.rearrange("(c p) t -> p c t", p=P)[:, :, ds(jp * PAN, PAN)]
            )
            for et in range(E // P):
                ps = psO.tile([P, PAN], F32, tag="psO")
                for dc in range(HPC):
                    nc.tensor.matmul(
                        ps,
                        lhsT=wo_sb[:, dc, ds(et * P, P)],
                        rhs=yl[:, dc, :],
                        start=(dc == 0),
                        stop=(dc == HPC - 1),
                    )
                ob = opool.tile([P, PAN], F32, tag="ob")
                nc.scalar.copy(ob, ps)
                nc.sync.dma_start(out[ds(et * P, P), ds(jp * PAN, PAN)], ob)


def round_f32r(a):
    """Round fp32 to the fp32r grid (11 mantissa bits, RNE) — matches the
    compiler's cast_fp32_to_fp32r bit-for-bit."""
    b = np.ascontiguousarray(a, dtype=np.float32).view(np.uint32).astype(np.uint64)
    lsb = (b >> 12) & 1
    b2 = (b + 0x7FF + lsb) & ~np.uint64(0xFFF)
    return b2.astype(np.uint32).view(np.float32).reshape(a.shape)


def make_tables():
    j = np.arange(0, D, 2, dtype=np.float64) / D          # (2j)/128
    inv_freq = 1.0 / (10000.0 ** j)                       # [64]
    t = np.arange(T, dtype=np.float64)
    fr = np.outer(t, inv_freq)                            # [T, 64]
    c2 = np.cos(2.0 * fr).T                               # [64, T]
    s2 = np.sin(2.0 * fr).T
    cos2 = np.concatenate([c2, c2], axis=0).astype(np.float32)
    sin2 = np.concatenate([s2, -s2], axis=0).astype(np.float32)
    return cos2, sin2


def make_masks():
    # mk[s, u] encodes the causal mask for diagonal-offset chunks:
    # chunk di uses columns [384 - 128*di, 384 - 128*di + 512), giving
    # 0 where (s + 128*di <= t) else NEG.
    s = np.arange(P)[:, None]
    u = np.arange(PAN + 384)[None, :]
    return np.where(s <= u - 384, 0.0, NEG).astype(np.float32)


def make_in_maps(x, wq, wk, wv, wo):
    cos2, sin2 = make_tables()
    masks = make_masks()
    in_maps = []
    for c in range(8):
        b, hh = c // 2, c % 2
        rows = slice(hh * HPC * D, (hh + 1) * HPC * D)
        in_maps.append({
            "xT": round_f32r(x[b].T),
            "wqT": round_f32r(wq[rows].T),
            "wkT": round_f32r(wk[rows].T),
            "wvT": round_f32r(wv[rows].T),
            "woT": round_f32r(wo[:, rows].T),
            "cos2": cos2,
            "sin2": sin2,
            "masks": masks,
        })
    return in_maps


_PROGRAM_CACHE = {}


def get_program():
    if "nc" not in _PROGRAM_CACHE:
        _PROGRAM_CACHE["nc"] = build_program()
    return _PROGRAM_CACHE["nc"]


def kernel(x, wq, wk, wv, wo, _results_hook=None):
    x = np.asarray(x, dtype=np.float32)
    wq = np.asarray(wq, dtype=np.float32)
    wk = np.asarray(wk, dtype=np.float32)
    wv = np.asarray(wv, dtype=np.float32)
    wo = np.asarray(wo, dtype=np.float32)

    nc = get_program()
    in_maps = make_in_maps(x, wq, wk, wv, wo)
    res = run_bass_kernel_spmd(nc, in_maps, list(range(8)))
    if _results_hook is not None:
        _results_hook(res)
    outs = [r["out"] for r in res.results]
    full = np.empty((B, T, E), dtype=np.float32)
    for b in range(B):
        full[b] = (outs[2 * b] + outs[2 * b + 1]).T
    return full


# revision 24
# speedup vs baseline: 1.0314x; 1.0314x over previous
"""Trainium2 Bass kernel for causal self-attention with doubled rotary.

Full-input contract: kernel(**inputs) takes the complete tensors
(x [4,2048,2048], wq/wk/wv/wo [2048,2048]) and returns [4,2048,2048] fp32.

Sharding: 8 cores = 4 batch elements x 2 head-halves (8 heads each).
Each core computes a partial output projection (its heads' columns of wo);
the host sums the two partials per batch element.

Per-core structure (engine streams execute in emission order, so independent
work is interleaved at emission time to keep the PE dense):
  - group g in 0..3 owns heads (2g, 2g+1): projections q/k/v (contraction
    over embd, fp32r matmuls), doubled-angle rotary on DVE (the reference
    applies rotary twice; R(t)^2 == R(2t)), all into double-buffered SBUF
    tiles.
  - attention pair g-1 is emitted interleaved with the projection of group
    g: QK^T computed transposed (ST[s,t]) so exp(ST) feeds the PV matmul
    directly with v as the stationary operand — no transposes. Causal
    diagonal chunks are sliced at the 128-column grid, with a single
    [128,128] triangular mask. Softmax denominators accumulate on DVE with
    the final add fused into the fp32r cast; an all-ones [128,128] matmul
    does the partition reduce + broadcast in one shot; full-width
    reciprocal; normalization rides the PSUM->SBUF copy of the PV output.
    yT goes to a DRAM spill for the output projection.
  - the last pair is interleaved with the first half of the output
    projection; the rest of the output projection follows.

All matmul operands are float32r (fp32 rounded to 11 mantissa bits, 4x the
fp32 PE rate). The host pre-rounds DRAM-fed operands; on-chip producers
(ACT exp, DVE copies/adds) round at their outputs.
"""

import os
import sys

for _p in ("/opt/trn_rl_repo", "/root/.axon_site/_ro/trn_rl_repo"):
    if os.path.isdir(_p) and _p not in sys.path:
        sys.path.insert(0, _p)

import numpy as np

import concourse.bass as bass
import concourse.mybir as mybir
from concourse import bacc
from concourse.bass import ds
from concourse.tile import TileContext
from concourse.bass_utils import run_bass_kernel_spmd

F32 = mybir.dt.float32
F32R = mybir.dt.float32r
BF16 = mybir.dt.bfloat16
FP16 = mybir.dt.float16

P = 128          # partitions / head dim
T = 2048         # sequence length
E = 2048         # embedding dim
B = 4
HPC = 8          # heads per core
D = 128          # head dim
PAN = 512        # attention t-panel width (PSUM bank limit for fp32)
NPAN = T // PAN  # 4
XPAN = 256       # projection t-panel width
NXP = T // XPAN  # 8
EO = E // P      # 16 contraction chunks for projections
NGRP = 4         # head pairs per core
NCH = T // P     # 16 s-chunks (also v t-tiles)
SCALE = 1.0 / float(np.sqrt(D))
NEG = -1.0e9

ADD = mybir.AluOpType.add
MULT = mybir.AluOpType.mult
EXP = mybir.ActivationFunctionType.Exp


def _zip_emit(*lists):
    """Emit thunks from several lists round-robin, proportionally."""
    lists = [list(l) for l in lists if l]
    if not lists:
        return
    total = max(len(l) for l in lists)
    idx = [0.0] * len(lists)
    step = [len(l) / total for l in lists]
    for _ in range(total):
        for li, l in enumerate(lists):
            idx[li] += step[li]
            while idx[li] >= 1.0 and l:
                l.pop(0)()
                idx[li] -= 1.0
    for l in lists:
        for f in l:
            f()


class Ctx:
    pass


def build_program():
    nc = bacc.Bacc()
    cx = Ctx()
    cx.nc = nc

    cx.xT = nc.declare_dram_parameter("xT", [E, T], F32R, isOutput=False)
    cx.wqT = nc.declare_dram_parameter("wqT", [E, HPC * D], F32R, isOutput=False)
    cx.wkT = nc.declare_dram_parameter("wkT", [E, HPC * D], F32R, isOutput=False)
    cx.wvT = nc.declare_dram_parameter("wvT", [E, HPC * D], F32R, isOutput=False)
    cx.woT = nc.declare_dram_parameter("woT", [HPC * D, E], F32R, isOutput=False)
    cx.cos2 = nc.declare_dram_parameter("cos2", [P, T], FP16, isOutput=False)
    cx.sin2 = nc.declare_dram_parameter("sin2", [P, T], FP16, isOutput=False)
    cx.mask = nc.declare_dram_parameter("mask", [P, P], BF16, isOutput=False)
    cx.out = nc.declare_dram_parameter("out", [E, T], F32, isOutput=True)
    cx.ytd = nc.dram_tensor("yt_scratch", [HPC * D, T], F32R)

    with TileContext(nc) as tc:
        cx.tc = tc
        with tc.tile_pool(name="const", bufs=1) as cpool:
            cx.mk = cpool.tile([P, P], BF16, tag="mk")
            nc.sync.dma_start(cx.mk, cx.mask[:, :])
            om_f = cpool.tile([P, P], F32, tag="om_f")
            nc.vector.memset(om_f, 1.0)
            cx.onesmat = cpool.tile([P, P], F32R, tag="onesmat")
            nc.scalar.copy(cx.onesmat, om_f)

            with (
                tc.tile_pool(name="ex", bufs=2) as expool,
                tc.tile_pool(name="dn", bufs=2) as dnpool,
                tc.tile_pool(name="dn1", bufs=1) as dn1pool,
                tc.tile_pool(name="yts", bufs=2) as ytpool,
                tc.tile_pool(name="psS", bufs=2, space="PSUM") as psS,
                tc.tile_pool(name="psY", bufs=2, space="PSUM") as psY,
                tc.tile_pool(name="psD", bufs=2, space="PSUM") as psD,
                tc.tile_pool(name="qk", bufs=2) as qkpool,
                tc.tile_pool(name="vp", bufs=2) as vpool,
            ):
                cx.expool, cx.dnpool, cx.dn1pool, cx.ytpool = (
                    expool, dnpool, dn1pool, ytpool)
                cx.psS, cx.psY, cx.psD = psS, psY, psD
                cx.qkpool, cx.vpool = qkpool, vpool
                cx.qkv = {}  # g -> (qT, kT, v_sb)

                with (
                    tc.tile_pool(name="tab", bufs=1) as tabpool,
                    tc.tile_pool(name="xp", bufs=2) as xpool,
                    tc.tile_pool(name="wp", bufs=1) as wpool,
                    tc.tile_pool(name="rot", bufs=1) as rotpool,
                    tc.tile_pool(name="sw", bufs=1) as swpool,
                    tc.tile_pool(name="psP", bufs=2, space="PSUM") as psP,
                ):
                    cx.xpool, cx.wpool = xpool, wpool
                    cx.rotpool, cx.swpool, cx.psP = rotpool, swpool, psP
                    cx.c2 = tabpool.tile([P, T], FP16, tag="c2")
                    nc.sync.dma_start(cx.c2, cx.cos2[:, :])
                    cx.s2 = tabpool.tile([P, T], FP16, tag="s2")
                    nc.sync.dma_start(cx.s2, cx.sin2[:, :])

                    for f in _proj_thunks(cx, 0):
                        f()
                    for g in range(1, NGRP):
                        _zip_emit(_proj_thunks(cx, g), _attn_thunks(cx, g - 1))

                with (
                    tc.tile_pool(name="wo", bufs=1) as wopool,
                    tc.tile_pool(name="yl", bufs=2) as ylpool,
                    tc.tile_pool(name="ob", bufs=3) as opool,
                    tc.tile_pool(name="psO", bufs=2, space="PSUM") as psO,
                ):
                    cx.wopool, cx.ylpool, cx.opool, cx.psO = (
                        wopool, ylpool, opool, psO)
                    # wo half 0 resident during the last attention pair
                    cx.wo_half = {}
                    _load_wo_half(cx, 0)
                    # outproj(jp) may only be emitted after pair-3 has
                    # finalized panel jp (it reads ytd rows for heads 6/7):
                    # interleave panel jp's outproj with panel jp+1's chunks.
                    panels = [_attn_thunks(cx, NGRP - 1, only_jp=jp)
                              for jp in range(NPAN)]
                    oproj0 = [_outproj_thunks(cx, 0, only_jp=jp)
                              for jp in range(NPAN)]
                    for f in panels[0]:
                        f()
                    for jp in range(1, NPAN):
                        _zip_emit(panels[jp], oproj0[jp - 1])
                    for f in oproj0[NPAN - 1]:
                        f()
                    _load_wo_half(cx, 1)
                    for f in _outproj_thunks(cx, 1):
                        f()

    nc.finalize()
    return nc


def _proj_thunks(cx, g):
    """Thunk list for group g's projections + rotary (no spill)."""
    nc = cx.nc
    thunks = []

    def start_group():
        wq_sb = cx.wpool.tile([P, EO, 2 * D], F32R, tag="wq")
        nc.sync.dma_start(
            wq_sb,
            cx.wqT.rearrange("(eo p) d -> p eo d", p=P)[:, :, ds(g * 2 * D, 2 * D)],
        )
        wk_sb = cx.wpool.tile([P, EO, 2 * D], F32R, tag="wk")
        nc.sync.dma_start(
            wk_sb,
            cx.wkT.rearrange("(eo p) d -> p eo d", p=P)[:, :, ds(g * 2 * D, 2 * D)],
        )
        wv_sb = cx.wpool.tile([P, EO, 2 * D], F32R, tag="wv")
        nc.sync.dma_start(
            wv_sb,
            cx.wvT.rearrange("(eo p) d -> p eo d", p=P)[:, :, ds(g * 2 * D, 2 * D)],
        )
        qT = cx.qkpool.tile([P, 2, T], F32R, tag="qT")
        kT = cx.qkpool.tile([P, 2, T], F32R, tag="kT")
        v_sb = cx.vpool.tile([P, NCH, 2 * D], F32R, tag="v")
        cx.qkv[g] = (qT, kT, v_sb)
        cx._w = (wq_sb, wk_sb, wv_sb)

    thunks.append(start_group)

    state = {}

    def load_panel(xj):
        def f():
            xp = cx.xpool.tile([P, EO, XPAN], F32R, tag="xp")
            nc.sync.dma_start(
                xp,
                cx.xT.rearrange("(eo p) t -> p eo t", p=P)[:, :, ds(xj * XPAN, XPAN)],
            )
            state[xj] = xp
        return f

    def qk_group(xj, wi, hl):
        def f():
            xp = state[xj]
            w_sb = cx._w[wi]
            dst = cx.qkv[g][wi]
            ps = cx.psP.tile([P, PAN], F32, tag="psP")
            psq = ps[:, :XPAN]
            for eo in range(EO):
                nc.tensor.matmul(
                    psq,
                    lhsT=w_sb[:, eo, ds(hl * D, D)],
                    rhs=xp[:, eo, :],
                    start=(eo == 0),
                    stop=(eo == EO - 1),
                )
            nc.vector.tensor_copy(dst[:, hl, ds(xj * XPAN, XPAN)], psq)
        return f

    def v_group(xj, tt):
        def f():
            xp = state[xj]
            wv_sb = cx._w[2]
            v_sb = cx.qkv[g][2]
            ps = cx.psP.tile([P, PAN], F32, tag="psP")
            psv = ps[:, : 2 * D]
            for eo in range(EO):
                nc.tensor.matmul(
                    psv,
                    lhsT=xp[:, eo, ds(tt * P, P)],
                    rhs=wv_sb[:, eo, :],
                    start=(eo == 0),
                    stop=(eo == EO - 1),
                )
            nc.vector.tensor_copy(v_sb[:, xj * (XPAN // P) + tt, :], psv)
        return f

    def rot_panel(src_i, hl, jp):
        def f():
            src = cx.qkv[g][src_i]
            sl = ds(jp * PAN, PAN)
            qsw = cx.swpool.tile([P, PAN], F32R, tag="qsw")
            nc.sync.dma_start(qsw[0:64, :], src[64:128, hl, sl])
            nc.sync.dma_start(qsw[64:128, :], src[0:64, hl, sl])
            tmp = cx.rotpool.tile([P, PAN], F32, tag="rtmp")
            nc.vector.tensor_tensor(tmp, qsw[:, :], cx.s2[:, sl], op=MULT)
            nc.vector.tensor_tensor(
                src[:, hl, sl], src[:, hl, sl], cx.c2[:, sl], op=MULT
            )
            nc.vector.tensor_tensor(src[:, hl, sl], src[:, hl, sl], tmp, op=ADD)
        return f

    for xj in range(NXP):
        thunks.append(load_panel(xj))
        for wi in range(2):
            for hl in range(2):
                thunks.append(qk_group(xj, wi, hl))
        for tt in range(XPAN // P):
            thunks.append(v_group(xj, tt))
        if xj % 2 == 1:
            jp = xj // 2
            for src_i in range(2):
                for hl in range(2):
                    thunks.append(rot_panel(src_i, hl, jp))
    return thunks


def _attn_thunks(cx, g, only_jp=None):
    """Thunk list for the attention of head pair g (heads 2g, 2g+1)."""
    nc = cx.nc
    thunks = []
    st8 = cx.__dict__.setdefault(f"_attn_state_{g}", {})

    def chunk(hl, jp, i):
        def f():
            qT, kT, v_sb = cx.qkv[g]
            nch = 4 * jp + 4
            if i == 0:
                ytp = cx.psY.tile([P, PAN], F32, tag="psY")
                den = cx.dnpool.tile([P, PAN], F32, tag="den")
                st8[(hl, jp)] = (ytp, den)
            ytp, den = st8[(hl, jp)]
            di = i - 4 * jp
            off = P * di if di > 0 else 0
            w = PAN - off
            st = cx.psS.tile([P, PAN], F32, tag="psS")
            stw = st[:, off:PAN]
            nc.tensor.matmul(
                stw,
                lhsT=kT[:, hl, ds(i * P, P)],
                rhs=qT[:, hl, ds(jp * PAN + off, w)],
                start=True,
                stop=True,
            )
            if di >= 0:
                nc.vector.tensor_tensor(
                    st[:, off:off + P], st[:, off:off + P], cx.mk, op=ADD
                )
            ex = cx.expool.tile([P, PAN], F32R, tag="ex")
            exw = ex[:, off:PAN]
            nc.scalar.activation(exw, stw, EXP, scale=SCALE)
            last = i == nch - 1
            if i == 0:
                nc.vector.tensor_copy(den, ex)
            elif last:
                denr = cx.dn1pool.tile([P, PAN], F32R, tag="denr")
                if off:
                    nc.vector.tensor_copy(denr[:, :off], den[:, :off])
                nc.vector.tensor_tensor(
                    denr[:, off:PAN], den[:, off:PAN], exw, op=ADD
                )
                st8[(hl, jp)] = (ytp, denr)
            else:
                nc.vector.tensor_tensor(
                    den[:, off:PAN], den[:, off:PAN], exw, op=ADD
                )
            nc.tensor.matmul(
                ytp[:, off:PAN],
                lhsT=v_sb[:, i, ds(hl * D, D)],
                rhs=exw,
                start=(i == 0),
                stop=last,
            )
        return f

    def finalize(hl, jp):
        def f():
            h = 2 * g + hl
            ytp, denr = st8.pop((hl, jp))
            rdbp = cx.psD.tile([P, PAN], F32, tag="psD")
            nc.tensor.matmul(
                rdbp, lhsT=cx.onesmat, rhs=denr, start=True, stop=True
            )
            rdb = cx.dn1pool.tile([P, PAN], F32, tag="rdb")
            nc.vector.reciprocal(rdb, rdbp)
            yts = cx.ytpool.tile([P, PAN], F32R, tag="yts")
            nc.vector.tensor_tensor(yts, ytp, rdb, op=MULT)
            nc.sync.dma_start(
                cx.ytd[ds(h * D, D), ds(jp * PAN, PAN)], yts
            )
        return f

    jps = range(NPAN) if only_jp is None else [only_jp]
    for jp in jps:
        nch = 4 * jp + 4
        for i in range(nch):
            for hl in range(2):
                thunks.append(chunk(hl, jp, i))
        for hl in range(2):
            thunks.append(finalize(hl, jp))
    return thunks


def _load_wo_half(cx, half):
    nc = cx.nc
    wo_sb = cx.wopool.tile([P, HPC, E // 2], F32R, tag="wo")
    nc.sync.dma_start(
        wo_sb,
        cx.woT.rearrange("(c p) e -> p c e", p=P)[:, :, ds(half * (E // 2), E // 2)],
    )
    cx.wo_half[half] = wo_sb


def _outproj_thunks(cx, half, only_jp=None):
    """Thunk list for the output projection over e-tiles of one wo half."""
    nc = cx.nc
    thunks = []
    yls = cx.__dict__.setdefault(f"_yl_state_{half}", {})

    def load_yl(jp):
        def f():
            yl = cx.ylpool.tile([P, HPC, PAN], F32R, tag="yl")
            nc.sync.dma_start(
                yl,
                cx.ytd.rearrange("(c p) t -> p c t", p=P)[:, :, ds(jp * PAN, PAN)],
            )
            yls[jp] = yl
        return f

    def etile(jp, et):
        def f():
            wo_sb = cx.wo_half[half]
            yl = yls[jp]
            ps = cx.psO.tile([P, PAN], F32, tag="psO")
            for dc in range(HPC):
                nc.tensor.matmul(
                    ps,
                    lhsT=wo_sb[:, dc, ds((et - half * 8) * P, P)],
                    rhs=yl[:, dc, :],
                    start=(dc == 0),
                    stop=(dc == HPC - 1),
                )
            ob = cx.opool.tile([P, PAN], F32, tag="ob")
            nc.scalar.copy(ob, ps)
            nc.sync.dma_start(
                cx.out[ds(et * P, P), ds(jp * PAN, PAN)], ob
            )
        return f

    jps = range(NPAN) if only_jp is None else [only_jp]
    for jp in jps:
        thunks.append(load_yl(jp))
        for et in range(half * 8, half * 8 + 8):
            thunks.append(etile(jp, et))
    return thunks


def round_f32r(a):
    """Round fp32 to the fp32r grid (11 mantissa bits, RNE) — matches the
    compiler's cast_fp32_to_fp32r bit-for-bit."""
    b = np.ascontiguousarray(a, dtype=np.float32).view(np.uint32).astype(np.uint64)
    lsb = (b >> 12) & 1
    b2 = (b + 0x7FF + lsb) & ~np.uint64(0xFFF)
    return b2.astype(np.uint32).view(np.float32).reshape(a.shape)


def make_tables():
    j = np.arange(0, D, 2, dtype=np.float64) / D
    inv_freq = 1.0 / (10000.0 ** j)
    t = np.arange(T, dtype=np.float64)
    fr = np.outer(t, inv_freq)                            # [T, 64]
    c2 = np.cos(2.0 * fr).T                               # [64, T]
    s2 = np.sin(2.0 * fr).T
    cos2 = np.concatenate([c2, c2], axis=0).astype(np.float16)
    sin2 = np.concatenate([s2, -s2], axis=0).astype(np.float16)
    return cos2, sin2


def make_mask():
    import ml_dtypes
    s = np.arange(P)[:, None]
    c = np.arange(P)[None, :]
    return np.where(s <= c, 0.0, NEG).astype(ml_dtypes.bfloat16)


def make_in_maps(x, wq, wk, wv, wo):
    cos2, sin2 = make_tables()
    mask = make_mask()
    in_maps = []
    for c in range(8):
        b, hh = c // 2, c % 2
        rows = slice(hh * HPC * D, (hh + 1) * HPC * D)
        in_maps.append({
            "xT": round_f32r(x[b].T),
            "wqT": round_f32r(wq[rows].T),
            "wkT": round_f32r(wk[rows].T),
            "wvT": round_f32r(wv[rows].T),
            "woT": round_f32r(wo[:, rows].T),
            "cos2": cos2,
            "sin2": sin2,
            "mask": mask,
        })
    return in_maps


_PROGRAM_CACHE = {}


def get_program():
    if "nc" not in _PROGRAM_CACHE:
        _PROGRAM_CACHE["nc"] = build_program()
    return _PROGRAM_CACHE["nc"]


def kernel(x, wq, wk, wv, wo, _results_hook=None):
    x = np.asarray(x, dtype=np.float32)
    wq = np.asarray(wq, dtype=np.float32)
    wk = np.asarray(wk, dtype=np.float32)
    wv = np.asarray(wv, dtype=np.float32)
    wo = np.asarray(wo, dtype=np.float32)

    nc = get_program()
    in_maps = make_in_maps(x, wq, wk, wv, wo)
    res = run_bass_kernel_spmd(nc, in_maps, list(range(8)))
    if _results_hook is not None:
        _results_hook(res)
    outs = [r["out"] for r in res.results]
    full = np.empty((B, T, E), dtype=np.float32)
    for b in range(B):
        full[b] = (outs[2 * b] + outs[2 * b + 1]).T
    return full
